# revision 1
# baseline (speedup 1.0000x reference)
"""BitLinear (BitNet b1.58) Trainium2 Bass kernel, token-sharded across 8 cores.

y = (round(clip(x/a_scale*127)) @ clip(round(W/w_scale),-1,1)^T) * w_scale*a_scale/127
  w_scale = mean(|W|)+eps (scalar), a_scale = max|x| per token + eps.

Shipped path (IMPL="v1", per core, SPMD x8): token-sharded bf16 kernel —
per 128-token block: DVE absmax reduce, ACT+DVE magic-add rounding to int8
values in bf16, SBUF->SBUF DMA transpose to k-major, 64 bf16 PE matmuls
j-outer interleaved across 4 PSUM banks x 512 (same lhsT feeds 4
consecutive matmuls; each bank's accumulation chain has 4-instruction
slack, which measured equal-or-faster and far less variance than
bank-sequential chains), ACT scaled drains, SWDGE y stores; W phase reads W
once (13 chunks resident), reduces sum|W| split DVE/ACT, ternary-quantizes
(clamp +-1.4999999 + round trick) and DMA-transposes into a resident bf16
[128, D/128, O] k-major rhs.

Alternate path (IMPL="v2", measured slower on HW despite better simulator
estimates; kept for reference):
- x sharded by tokens (16384/8 = 2048/core); W replicated (each core needs all
  of W quantized for its tokens' matmul).
- fp8 DoubleRow matmul: int8 activations a split exactly as a = 16h + l
  (h = round(a/16) in [-8,8], l in [-9,9]); 16h and l are exact in fp8e4m3,
  ternary weights exact in fp8. The PE DoubleRow perf mode contracts two
  adjacent 128-deep k-planes per instruction at 0.5 cycles/row, halving
  matmul time vs bf16; fp32 PSUM accumulation keeps everything bit-exact
  integer arithmetic, so the result equals the int8 reference matmul.
- All rounding via the +-1.5*2^23 magic-add trick (RNE). The h/l split is
  computed in five full-tile passes balanced across ACT (t2 = x*s/16+C,
  t1 = x*s+C), Pool/GPSIMD (h16 = 16*t2-16C, v = -16*t2+15C, l = t1+v), with
  the per-token absmax reduce on DVE (DVE passes are the HW bottleneck —
  measured ~2.5us/pass vs 1.2 ACT / 1.34 Pool for [128,2048]).
- Transposes on the PE (128x128 bf16 transpose via identity matmul into
  PSUM), with fp8-casting copybacks split ACT/DVE. No DMA transposes.
- W phase: single read (12 chunks resident + 4 streamed/re-read), sum|W|
  reduces split DVE/ACT, global partition reduce on GPSIMD, ternary
  quantize (clamp +-1.4999999 + round trick) on Pool/DVE, PE transposes
  into a resident fp8 [128, D/128, O] k-major rhs; DoubleRow pairs are
  adjacent k planes (no weight duplication). W loads ride the SWDGE queue,
  x loads the SP HWDGE queue, y stores alternate SP/ACT HWDGE queues.
- Main loop software-pipelined: quant stage A runs 2-3 token blocks ahead,
  transpose stage B one block ahead of its 16 DoubleRow matmuls (4 PSUM
  banks x 512 cols); scaled drains alternate ACT/DVE.
"""

import sys

sys.path.insert(0, "/opt/trn_rl_repo")

import numpy as np

import concourse.bacc as bacc
import concourse.bass as bass
import concourse.bass_isa as bass_isa
import concourse.mybir as mybir
import concourse.tile as tile

P = 128
MM_N = 512  # free-dim per matmul (one PSUM bank)
EXP23 = 12582912.0  # 1.5*2**23; v + C - C == RNE round for |v| < 2**22
EPS = 1e-8
CLIP_HI = 1.4999999  # largest f32 < 1.5; clamp-then-round == clip(round(.),-1,1)
N_CORES = 8
F32 = mybir.dt.float32
BF16 = mybir.dt.bfloat16
ALU = mybir.AluOpType
AFT = mybir.ActivationFunctionType
AX = mybir.AxisListType


def emit_bitlinear(
    tc: "tile.TileContext",
    y: "bass.AP",
    xs: "bass.AP",
    w: "bass.AP",
    repeat: int = 1,
    variant: str = "full",
):
    nc = tc.nc
    T, D = xs.shape
    O = w.shape[0]
    TB, NI, NR = T // P, D // P, O // P
    NOB = O // MM_N
    no_wphase = variant in ("no_wphase", "mm1")
    mm1 = variant == "mm1"

    from contextlib import ExitStack

    for _rep in range(repeat):
        _emit_bitlinear_once(tc, y, xs, w, variant, _rep)


def _emit_bitlinear_once(
    tc: "tile.TileContext",
    y: "bass.AP",
    xs: "bass.AP",
    w: "bass.AP",
    variant: str,
    rep: int = 0,
):
    nc = tc.nc
    T, D = xs.shape
    O = w.shape[0]
    TB, NI, NR = T // P, D // P, O // P
    NOB = O // MM_N
    no_wphase = variant in ("no_wphase", "mm1")
    mm1 = variant == "mm1"
    qsep = variant == "jouterq"  # W loads via SWDGE, y via HWDGE queues
    # jouter4: 4 early chains (needs one more streamed W chunk + aqte slots)
    NRES = NR - 4 if variant == "jouter4" else NR - 3

    from contextlib import ExitStack

    with ExitStack() as ctx:
        small = ctx.enter_context(tc.tile_pool(name=f"small{rep}", bufs=1))
        wqt_pool = ctx.enter_context(tc.tile_pool(name=f"wqt{rep}", bufs=1))
        ps_pool = ctx.enter_context(
            tc.tile_pool(name=f"psp{rep}", bufs=8, space="PSUM")
        )
        sc_pool = ctx.enter_context(tc.tile_pool(name=f"scp{rep}", bufs=4))
        # early-chain pools live across the whole kernel but are small; the
        # early aqT tiles must outlive the W phase (PE reads them later)
        aqte_pool = ctx.enter_context(tc.tile_pool(
            name=f"aqte{rep}", bufs=4 if variant == "jouter4" else 2))

        wqT = wqt_pool.tile([P, NI, O], BF16)
        w_scale = small.tile([P, 1], F32)

        def x_chain(tb, xpool_, aq_pool_, aqt_pool_):
            xt = xpool_.tile([P, D], F32, tag="x", name=f"x_{tb}")
            nc.sync.dma_start(xt[:], xs[tb * P : (tb + 1) * P, :])
            amax = sc_pool.tile([P, 1], F32, tag="amax", name=f"amax_{tb}")
            nc.vector.tensor_reduce(
                amax[:], xt[:], axis=AX.X, op=ALU.max, apply_absolute_value=True
            )
            a_eps = sc_pool.tile([P, 1], F32, tag="aeps", name=f"aeps_{tb}",
                                 bufs=TB)
            nc.vector.tensor_scalar_add(a_eps[:], amax[:], EPS)
            rcp = sc_pool.tile([P, 1], F32, tag="rcp", name=f"rcp_{tb}")
            nc.vector.reciprocal(rcp[:], a_eps[:])
            inv127 = sc_pool.tile([P, 1], F32, tag="i127", name=f"i127_{tb}")
            nc.vector.tensor_scalar_mul(inv127[:], rcp[:], 127.0)
            # round in place: xt <- xt*inv127 + C (ACT), then -C -> bf16 (DVE)
            nc.scalar.activation(
                xt[:], xt[:], AFT.Copy, bias=EXP23, scale=inv127[:]
            )
            aq = aq_pool_.tile([P, D], BF16, tag="aq", name=f"aq_{tb}")
            nc.vector.tensor_scalar_add(aq[:], xt[:], -EXP23)
            # aqT[p, j, t] = aq[t, j*128 + p]
            aqT = aqt_pool_.tile([P, NI, P], BF16, tag="aqT", name=f"aqT_{tb}")
            nc.scalar.dma_start(aqT[:], aq[:], transpose=True)
            return aqT, a_eps

        early = {}

        # ---- W phase: single read; NRES tiles resident, 3 streamed through
        # the xe pool, which also hosts two early token chains.
        if no_wphase:
            nc.gpsimd.memset(wqT[:], 1.0)
            nc.gpsimd.memset(w_scale[:], 0.01)
        else:
            with tc.tile_pool(name=f"wres{rep}", bufs=NRES) as wres, \
                 tc.tile_pool(name=f"wqrp{rep}", bufs=2) as wqrp, \
                 tc.tile_pool(name=f"xe{rep}", bufs=2) as xe_pool, \
                 tc.tile_pool(name=f"aqe{rep}", bufs=1) as aqe_pool:
                wsums = small.tile([P, NR], F32)
                wts = []

                def w_load(r, pool, tag, keep, name_pfx="wt"):
                    wt = pool.tile([P, D], F32, tag=tag,
                                   name=f"{name_pfx}_{r}")
                    weng = nc.gpsimd if qsep else nc.sync
                    weng.dma_start(wt[:], w[r * P : (r + 1) * P, :])
                    if keep:
                        wts.append(wt)
                        return wt
                    if r % 2 == 0:
                        nc.vector.tensor_reduce(
                            wsums[:, r : r + 1], wt[:], axis=AX.X, op=ALU.add,
                            apply_absolute_value=True,
                        )
                    else:
                        trash = wqrp.tile([P, D], BF16, tag="wqr",
                                          name=f"trash_{r}")
                        nc.scalar.activation(
                            trash[:], wt[:], AFT.Abs,
                            accum_out=wsums[:, r : r + 1],
                        )
                    wts.append(wt)
                    return wt

                for r in range(NRES):
                    w_load(r, wres, "wt", keep=False)
                # two early token chains through the xe/aqe pools; the last 3
                # W tiles are reduce-only here and re-read before quantize
                early[0] = x_chain(0, xe_pool, aqe_pool, aqte_pool)
                for r in range(NRES, NR):
                    wt = w_load(r, xe_pool, "x", keep=True, name_pfx="wta")
                    wts.pop()
                    if r % 2 == 0:
                        nc.vector.tensor_reduce(
                            wsums[:, r : r + 1], wt[:], axis=AX.X, op=ALU.add,
                            apply_absolute_value=True,
                        )
                    else:
                        trash = wqrp.tile([P, D], BF16, tag="wqr",
                                          name=f"trash_{r}")
                        nc.scalar.activation(
                            trash[:], wt[:], AFT.Abs,
                            accum_out=wsums[:, r : r + 1],
                        )
                    if r == NRES:
                        early[1] = x_chain(1, xe_pool, aqe_pool, aqte_pool)

                wsum1 = small.tile([P, 1], F32)
                nc.vector.tensor_reduce(
                    wsum1[:], wsums[:], axis=AX.X, op=ALU.add
                )
                wsum_all = small.tile([P, 1], F32)
                nc.gpsimd.partition_all_reduce(
                    wsum_all[:], wsum1[:], channels=P,
                    reduce_op=bass_isa.ReduceOp.add,
                )
                nc.vector.tensor_scalar(
                    w_scale[:], wsum_all[:], 1.0 / (O * D), EPS, op0=ALU.mult,
                    op1=ALU.add,
                )
                inv_w = small.tile([P, 1], F32)
                nc.vector.reciprocal(inv_w[:], w_scale[:])

                # re-read the streamed tiles now that w_scale is known
                for r in range(NRES, NR):
                    w_load(r, xe_pool, "x", keep=True, name_pfx="wtb")

                # in-place ternary quantize; streamed chunks first under
                # jouter4 so their xe slots free for the extra early chains
                if variant == "jouter4":
                    qorder = list(range(NRES, NR)) + list(range(NRES))
                else:
                    qorder = list(range(NR))
                for r in qorder:
                    u = wts[r]
                    eng = nc.gpsimd if r % 2 == 0 else nc.vector
                    eng.tensor_scalar(
                        u[:], u[:], inv_w[:], CLIP_HI, op0=ALU.mult,
                        op1=ALU.min,
                    )
                    eng.tensor_scalar(
                        u[:], u[:], -CLIP_HI, EXP23, op0=ALU.max, op1=ALU.add
                    )
                    wqr = wqrp.tile([P, D], BF16, tag="wqr", name=f"wqr_{r}")
                    if r % 2 == 0:
                        nc.scalar.activation(
                            wqr[:], u[:], AFT.Copy, bias=-EXP23
                        )
                    else:
                        nc.gpsimd.tensor_scalar_add(wqr[:], u[:], -EXP23)
                    nc.sync.dma_start(
                        wqT[:, :, r * P : (r + 1) * P], wqr[:], transpose=True
                    )
                    if variant == "jouter4" and r >= NR - 2:
                        k = 2 + (r - (NR - 2))
                        if k < TB and k not in early:
                            early[k] = x_chain(
                                k, xe_pool, aqe_pool, aqte_pool
                            )

        # ---- token pipeline pools (reuse the released W-phase SBUF).
        # "jouterslim" probes the pool-trim needed to free ~52KB/partition
        # for a future cross-rep W-prefetch ring (not shipped).
        slim = variant == "jouterslim"
        xpool = ctx.enter_context(
            tc.tile_pool(name=f"xp{rep}", bufs=3 if slim else 6))
        aq_pool = ctx.enter_context(
            tc.tile_pool(name=f"aqp{rep}", bufs=2 if slim else 4))
        aqt_pool = ctx.enter_context(
            tc.tile_pool(name=f"aqtp{rep}", bufs=4 if slim else 8))
        ypool = ctx.enter_context(
            tc.tile_pool(name=f"yp{rep}", bufs=4 if slim else 6))

        es_cache = {}

        def get_es(tb, a_eps):
            if tb not in es_cache:
                es = sc_pool.tile([P, 1], F32, tag="es", name=f"es_{tb}",
                                  bufs=8)
                nc.vector.tensor_scalar(
                    es[:], a_eps[:], w_scale[:], 1.0 / 127.0, op0=ALU.mult,
                    op1=ALU.mult,
                )
                es_cache[tb] = es
            return es_cache[tb]

        def drain(tb, ob, ps, es):
            ysb = ypool.tile([P, MM_N], F32, tag="y", name=f"y_{tb}_{ob}")
            nc.scalar.activation(ysb[:], ps[:], AFT.Copy, scale=es[:])
            if qsep:
                yeng = nc.sync if ob % 2 == 0 else nc.scalar
            else:
                yeng = nc.gpsimd
            yeng.dma_start(
                y[tb * P : (tb + 1) * P, ob * MM_N : (ob + 1) * MM_N], ysb[:]
            )

        def mm_quarter(tb, ob, aqT, a_eps):
            es = get_es(tb, a_eps)
            ps = ps_pool.tile([P, MM_N], F32, tag="ps", name=f"ps_{tb}_{ob}")
            NJ = 1 if mm1 else NI
            for j in range(NJ):
                nc.tensor.matmul(
                    ps[:],
                    lhsT=aqT[:, j, :],
                    rhs=wqT[:, j, ob * MM_N : (ob + 1) * MM_N],
                    start=(j == 0),
                    stop=(j == NJ - 1),
                )
            drain(tb, ob, ps, es)

        def mm_jouter(tb, aqT, a_eps):
            es = get_es(tb, a_eps)
            pss = [
                ps_pool.tile([P, MM_N], F32, tag="ps", name=f"ps_{tb}_{ob}")
                for ob in range(NOB)
            ]
            for j in range(NI):
                for ob in range(NOB):
                    nc.tensor.matmul(
                        pss[ob][:],
                        lhsT=aqT[:, j, :],
                        rhs=wqT[:, j, ob * MM_N : (ob + 1) * MM_N],
                        start=(j == 0),
                        stop=(j == NI - 1),
                    )
            for ob in range(NOB):
                drain(tb, ob, pss[ob], es)

        # ---- main loop. Hybrid ordering for j-outer variants: the first
        # two token blocks run ob-outer (their 16-matmul chains need only
        # wqT chunks 4ob..4ob+3, so the PE starts while later W chunks are
        # still quantizing/transposing — deps are range-tracked); steady
        # state runs j-outer for its accumulation-chain slack.
        jouter = variant in ("jouter", "jouterq", "jouter4", "jouterslim")
        for tb in range(TB):
            if tb in early:
                aqT, a_eps = early[tb]
            else:
                aqT, a_eps = x_chain(tb, xpool, aq_pool, aqt_pool)
            if jouter and tb >= 2:
                mm_jouter(tb, aqT, a_eps)
            else:
                for ob in range(NOB):
                    mm_quarter(tb, ob, aqT, a_eps)


FP8 = mybir.dt.float8e4
EXP23_16 = 16.0 * EXP23
EXP23_15 = 15.0 * EXP23


def emit_bitlinear_v2(
    tc: "tile.TileContext",
    y: "bass.AP",
    xs: "bass.AP",
    w: "bass.AP",
    repeat: int = 1,
    variant: str = "full",
    mm: str = "dr",
):
    for rep in range(repeat):
        _emit_v2_once(tc, y, xs, w, variant, mm, rep)


def _emit_v2_once(
    tc: "tile.TileContext",
    y: "bass.AP",
    xs: "bass.AP",
    w: "bass.AP",
    variant: str,
    mm: str,
    rep: int,
):
    from contextlib import ExitStack

    from concourse import masks

    nc = tc.nc
    T, D = xs.shape
    O = w.shape[0]
    TB, NI, NR = T // P, D // P, O // P
    NOB = O // MM_N
    dr = mm == "dr"
    mm1 = variant == "mm1"
    NRES = NR - 4  # W tiles kept resident; rest streamed + re-read
    N_EARLY = min(4, TB)

    with ExitStack() as ctx:
        small = ctx.enter_context(tc.tile_pool(name=f"v2small{rep}", bufs=1))
        wq_pool = ctx.enter_context(tc.tile_pool(name=f"v2wq{rep}", bufs=1))
        ps_mm = ctx.enter_context(
            tc.tile_pool(name=f"v2psmm{rep}", bufs=4, space="PSUM")
        )
        ps_tr = ctx.enter_context(
            tc.tile_pool(name=f"v2pstr{rep}", bufs=3, space="PSUM")
        )
        sc_pool = ctx.enter_context(tc.tile_pool(name=f"v2scp{rep}", bufs=4))
        aqte_pool = ctx.enter_context(
            tc.tile_pool(name=f"v2aqte{rep}", bufs=N_EARLY)
        )

        ident = small.tile([P, P], BF16)
        masks.make_identity(nc, ident[:])
        # wq8: ternary weights, k-major [P(i), NI(j), O]; DR pairs are
        # adjacent j planes. bf16 path stores bf16 instead.
        wq8 = wq_pool.tile([P, NI, O], FP8 if dr else BF16)
        w_scale = small.tile([P, 1], F32)

        def x_stage_a(tb, xpool_, t2pool_, hlpool_):
            """DMA + absmax + quant passes; returns tiles for stage b."""
            xt = xpool_.tile([P, D], F32, tag="x", name=f"x_{tb}")
            nc.sync.dma_start(xt[:], xs[tb * P : (tb + 1) * P, :])
            amax = sc_pool.tile([P, 1], F32, tag="amax", name=f"amax_{tb}")
            nc.vector.tensor_reduce(
                amax[:], xt[:], axis=AX.X, op=ALU.max, apply_absolute_value=True
            )
            a_eps = sc_pool.tile(
                [P, 1], F32, tag="aeps", name=f"aeps_{tb}", bufs=TB
            )
            nc.vector.tensor_scalar_add(a_eps[:], amax[:], EPS)
            rcp = sc_pool.tile([P, 1], F32, tag="rcp", name=f"rcp_{tb}")
            nc.vector.reciprocal(rcp[:], a_eps[:])
            s = sc_pool.tile([P, 1], F32, tag="s", name=f"s_{tb}")
            nc.vector.tensor_scalar_mul(s[:], rcp[:], 127.0)
            if dr:
                s16 = sc_pool.tile([P, 1], F32, tag="s16", name=f"s16_{tb}")
                nc.vector.tensor_scalar_mul(s16[:], rcp[:], 127.0 / 16.0)
                # t2 = x*s/16 + C  (== C + h exactly, h = round(a/16))
                t2 = t2pool_.tile([P, D], F32, tag="t2", name=f"t2_{tb}")
                nc.scalar.activation(
                    t2[:], xt[:], AFT.Copy, bias=EXP23, scale=s16[:]
                )
                # t1 = x*s + C in place (== C + a exactly)
                nc.scalar.activation(
                    xt[:], xt[:], AFT.Copy, bias=EXP23, scale=s[:]
                )
                # h16 = 16h = 16*t2 - 16C (bf16); on Pool to offload ACT/DVE
                h16 = hlpool_.tile([P, D], BF16, tag="h", name=f"h_{tb}")
                nc.gpsimd.tensor_scalar(
                    h16[:], t2[:], 16.0, -EXP23_16, op0=ALU.mult, op1=ALU.add
                )
                # v = -16*t2 + 15C = -C - 16h, in place over t2 (Pool)
                nc.gpsimd.tensor_scalar(
                    t2[:], t2[:], -16.0, EXP23_15, op0=ALU.mult, op1=ALU.add
                )
                # l = t1 + v = a - 16h (bf16) (Pool)
                l16 = hlpool_.tile([P, D], BF16, tag="l", name=f"l_{tb}")
                nc.gpsimd.tensor_tensor(l16[:], xt[:], t2[:], op=ALU.add)
                return (tb, ((0, h16), (1, l16)), a_eps, 2)
            else:
                nc.scalar.activation(
                    xt[:], xt[:], AFT.Copy, bias=EXP23, scale=s[:]
                )
                aq = hlpool_.tile([P, D], BF16, tag="h", name=f"aq_{tb}")
                nc.gpsimd.tensor_scalar_add(aq[:], xt[:], -EXP23)
                return (tb, ((0, aq),), a_eps, 1)

        def x_stage_b(st, aqt_pool_):
            """PE transposes (bf16) -> psum -> cast copyback (rotated)."""
            tb, srcs, a_eps, npair = st
            aqT = aqt_pool_.tile(
                [P, npair, NI, P], FP8 if dr else BF16, tag="aqT",
                name=f"aqT_{tb}",
            )
            cb_engs = (nc.scalar, nc.vector, nc.scalar, nc.vector)
            for half, src in srcs:
                for g in range(2):
                    pst = ps_tr.tile(
                        [P, 8 * P], BF16, tag="pst",
                        name=f"pst_{tb}_{half}_{g}",
                    )
                    for k in range(8):
                        j = g * 8 + k
                        nc.tensor.transpose(
                            pst[:, k * P : (k + 1) * P],
                            src[:, j * P : (j + 1) * P],
                            ident[:],
                        )
                    eng = cb_engs[(half * 2 + g) % len(cb_engs)]
                    dst = aqT[:, half, g * 8 : (g + 1) * 8, :]
                    if eng is nc.scalar:
                        nc.scalar.activation(dst, pst[:], AFT.Copy)
                    else:
                        eng.tensor_copy(dst, pst[:])
            return aqT, a_eps

        stage_a = {}
        done_b = {}

        # ---- W phase (W loads on the scalar DMA queue; x on sync queue)
        if variant == "no_wphase":
            nc.vector.memset(wq8[:], 1.0)
            nc.vector.memset(w_scale[:], 0.01)
        else:
            with tc.tile_pool(name=f"v2wres{rep}", bufs=NRES) as wres, \
                 tc.tile_pool(name=f"v2wqr{rep}", bufs=2) as wqrp, \
                 tc.tile_pool(name=f"v2xe{rep}", bufs=3) as xe_pool, \
                 tc.tile_pool(name=f"v2ep{rep}", bufs=1) as early_pool:
                wsums = small.tile([P, NR], F32)
                wts = []

                def w_reduce(r, wt):
                    if r % 2 == 0:
                        nc.vector.tensor_reduce(
                            wsums[:, r : r + 1], wt[:], axis=AX.X, op=ALU.add,
                            apply_absolute_value=True,
                        )
                    else:
                        trash = wqrp.tile([P, D], BF16, tag="wqr",
                                          name=f"trash_{r}")
                        nc.scalar.activation(
                            trash[:], wt[:], AFT.Abs,
                            accum_out=wsums[:, r : r + 1],
                        )

                n_a = 0
                for r in range(NRES):
                    wt = wres.tile([P, D], F32, tag="wt", name=f"wt_{r}")
                    nc.gpsimd.dma_start(wt[:], w[r * P : (r + 1) * P, :])
                    w_reduce(r, wt)
                    wts.append(wt)
                    if r % 3 == 2 and n_a < N_EARLY:
                        stage_a[n_a] = x_stage_a(
                            n_a, early_pool, early_pool, early_pool
                        )
                        n_a += 1
                for r in range(NRES, NR):
                    wt = xe_pool.tile([P, D], F32, tag="x", name=f"wta_{r}")
                    nc.gpsimd.dma_start(wt[:], w[r * P : (r + 1) * P, :])
                    w_reduce(r, wt)
                while n_a < N_EARLY:
                    stage_a[n_a] = x_stage_a(
                        n_a, early_pool, early_pool, early_pool
                    )
                    n_a += 1

                wsum1 = small.tile([P, 1], F32)
                nc.vector.tensor_reduce(
                    wsum1[:], wsums[:], axis=AX.X, op=ALU.add
                )
                wsum_all = small.tile([P, 1], F32)
                nc.gpsimd.partition_all_reduce(
                    wsum_all[:], wsum1[:], channels=P,
                    reduce_op=bass_isa.ReduceOp.add,
                )
                nc.vector.tensor_scalar(
                    w_scale[:], wsum_all[:], 1.0 / (O * D), EPS, op0=ALU.mult,
                    op1=ALU.add,
                )
                inv_w = small.tile([P, 1], F32)
                nc.vector.reciprocal(inv_w[:], w_scale[:])

                done_b[0] = x_stage_b(stage_a.pop(0), aqte_pool)

                for r in range(NRES, NR):
                    wt = xe_pool.tile([P, D], F32, tag="x", name=f"wtb_{r}")
                    nc.gpsimd.dma_start(wt[:], w[r * P : (r + 1) * P, :])
                    wts.append(wt)

                for r in range(NR):
                    u = wts[r]
                    eng = nc.gpsimd if r % 2 == 0 else nc.vector
                    eng.tensor_scalar(
                        u[:], u[:], inv_w[:], CLIP_HI, op0=ALU.mult,
                        op1=ALU.min,
                    )
                    eng.tensor_scalar(
                        u[:], u[:], -CLIP_HI, EXP23, op0=ALU.max, op1=ALU.add
                    )
                    wqr = wqrp.tile([P, D], BF16, tag="wqr", name=f"wqr_{r}")
                    if r % 2 == 0:
                        nc.scalar.activation(
                            wqr[:], u[:], AFT.Copy, bias=-EXP23
                        )
                    else:
                        nc.gpsimd.tensor_scalar_add(wqr[:], u[:], -EXP23)
                    # PE transpose wqr [o,i] -> wq8[:, :, r*128...] (i major)
                    for g in range(2):
                        pst = ps_tr.tile(
                            [P, 8 * P], BF16, tag="pst",
                            name=f"wpst_{r}_{g}",
                        )
                        for k in range(8):
                            j = g * 8 + k
                            nc.tensor.transpose(
                                pst[:, k * P : (k + 1) * P],
                                wqr[:, j * P : (j + 1) * P],
                                ident[:],
                            )
                        dst = wq8[:, g * 8 : (g + 1) * 8, r * P : (r + 1) * P]
                        src8 = pst[:].rearrange("p (j o) -> p j o", j=8)
                        if r % 2 == 0:
                            nc.vector.tensor_copy(dst, src8)
                        else:
                            nc.scalar.activation(dst, src8, AFT.Copy)
                    if r % 4 == 1 and (r // 4 + 1) < N_EARLY:
                        k = r // 4 + 1
                        done_b[k] = x_stage_b(stage_a.pop(k), aqte_pool)
                while stage_a:
                    k = min(stage_a)
                    done_b[k] = x_stage_b(stage_a.pop(k), aqte_pool)

        # ---- main token pipeline (software pipelined: A two ahead,
        # B one ahead of the matmul consumer)
        xpool = ctx.enter_context(tc.tile_pool(name=f"v2xp{rep}", bufs=4))
        t2pool = ctx.enter_context(tc.tile_pool(name=f"v2t2{rep}", bufs=4))
        hlpool = ctx.enter_context(tc.tile_pool(name=f"v2hl{rep}", bufs=4))
        aqt_pool = ctx.enter_context(tc.tile_pool(name=f"v2aqt{rep}", bufs=4))
        ypool = ctx.enter_context(tc.tile_pool(name=f"v2yp{rep}", bufs=6))

        es_cache = {}

        def mm_quarter(tb, ob, aqT, a_eps):
            if tb not in es_cache:
                es = sc_pool.tile([P, 1], F32, tag="es", name=f"es_{tb}",
                                  bufs=8)
                nc.vector.tensor_scalar(
                    es[:], a_eps[:], w_scale[:], 1.0 / 127.0, op0=ALU.mult,
                    op1=ALU.mult,
                )
                es_cache[tb] = es
            es = es_cache[tb]
            ps = ps_mm.tile([P, MM_N], F32, tag="ps", name=f"ps_{tb}_{ob}")
            sl = slice(ob * MM_N, (ob + 1) * MM_N)
            if dr:
                NJP = 1 if mm1 else NI // 2
                for half in range(2):
                    for jp in range(NJP):
                        nc.tensor.matmul(
                            ps[:],
                            lhsT=aqT[:, half, 2 * jp : 2 * jp + 2, :],
                            rhs=wq8[:, 2 * jp : 2 * jp + 2, sl],
                            start=(half == 0 and jp == 0),
                            stop=(half == 1 and jp == NJP - 1),
                            perf_mode=mybir.MatmulPerfMode.DoubleRow,
                        )
            else:
                NJ = 1 if mm1 else NI
                for j in range(NJ):
                    nc.tensor.matmul(
                        ps[:],
                        lhsT=aqT[:, 0, j, :],
                        rhs=wq8[:, j, sl],
                        start=(j == 0),
                        stop=(j == NJ - 1),
                    )
            ysb = ypool.tile([P, MM_N], F32, tag="y", name=f"y_{tb}_{ob}")
            if ob % 2 == 0:
                nc.scalar.activation(ysb[:], ps[:], AFT.Copy, scale=es[:])
            else:
                nc.vector.tensor_scalar_mul(ysb[:], ps[:], es[:])
            yeng = nc.sync if ob % 2 == 0 else nc.scalar
            yeng.dma_start(
                y[tb * P : (tb + 1) * P, ob * MM_N : (ob + 1) * MM_N], ysb[:]
            )

        for tb in range(TB):
            for ta in (tb + 2, tb + 3):
                if ta < TB and ta not in done_b and ta not in stage_a:
                    stage_a[ta] = x_stage_a(ta, xpool, t2pool, hlpool)
            tbb = tb + 1
            if tbb < TB and tbb not in done_b:
                done_b[tbb] = x_stage_b(stage_a.pop(tbb), aqt_pool)
            if tb not in done_b:
                done_b[tb] = x_stage_b(stage_a.pop(tb), aqt_pool)
            aqT, a_eps = done_b[tb]
            for ob in range(NOB):
                mm_quarter(tb, ob, aqT, a_eps)
            del done_b[tb]
_NC_CACHE: dict = {}


def _get_nc(
    T: int,
    D: int,
    O: int,
    repeat: int = 1,
    variant: str | None = None,
    impl: str | None = None,
    mm: str = "dr",
) -> "bass.Bass":
    if impl is None:
        impl = IMPL
    if variant is None:
        variant = V1_VARIANT if impl == "v1" else "full"
    key = (T, D, O, repeat, variant, impl, mm)
    if key not in _NC_CACHE:
        nc = bacc.Bacc("TRN2", target_bir_lowering=False, debug=False)
        xs = nc.dram_tensor("xs", [T, D], F32, kind="ExternalInput").ap()
        w = nc.dram_tensor("w", [O, D], F32, kind="ExternalInput").ap()
        y = nc.dram_tensor("y", [T, O], F32, kind="ExternalOutput").ap()
        with tile.TileContext(nc) as tc:
            if impl == "v1":
                emit_bitlinear(tc, y, xs, w, repeat=repeat, variant=variant)
            else:
                emit_bitlinear_v2(
                    tc, y, xs, w, repeat=repeat, variant=variant, mm=mm
                )
        nc.compile()
        _NC_CACHE[key] = nc
    return _NC_CACHE[key]


IMPL = "v1"
MM = "dr"
V1_VARIANT = "jouter"


def kernel(
    x: np.ndarray, weight: np.ndarray, _trace: bool = False, _repeat: int = 1
):
    from concourse.bass_utils import run_bass_kernel_spmd

    x = np.asarray(x, dtype=np.float32)
    weight = np.ascontiguousarray(np.asarray(weight, dtype=np.float32))
    B, S, D = x.shape
    O = weight.shape[0]
    tokens = B * S
    Tc = tokens // N_CORES
    xf = np.ascontiguousarray(x.reshape(tokens, D))

    nc = _get_nc(Tc, D, O, repeat=_repeat, impl=IMPL, mm=MM)
    in_maps = [
        {"xs": np.ascontiguousarray(xf[c * Tc : (c + 1) * Tc]), "w": weight}
        for c in range(N_CORES)
    ]
    res = run_bass_kernel_spmd(
        nc, in_maps, list(range(N_CORES)), trace=_trace
    )
    out = np.concatenate([res.results[c]["y"] for c in range(N_CORES)], axis=0)
    out = out.reshape(B, S, O)
    if _trace:
        return out, res
    return out



# revision 11
# speedup vs baseline: 3.0518x; 3.0518x over previous
"""BitLinear (BitNet b1.58) Trainium2 Bass kernel, token-sharded across 8 cores.

y = (round(clip(x/a_scale*127)) @ clip(round(W/w_scale),-1,1)^T) * w_scale*a_scale/127
  w_scale = mean(|W|)+eps (scalar), a_scale = max|x| per token + eps.

Shipped path (IMPL="v1", per core, SPMD x8): token-sharded bf16 kernel —
per 128-token block: DVE absmax reduce, ACT+DVE magic-add rounding to int8
values in bf16, SBUF->SBUF DMA transpose to k-major, 64 bf16 PE matmuls
j-outer interleaved across 4 PSUM banks x 512 (same lhsT feeds 4
consecutive matmuls; each bank's accumulation chain has 4-instruction
slack, which measured equal-or-faster and far less variance than
bank-sequential chains), ACT scaled drains, SWDGE y stores; W phase reads W
once (13 chunks resident), reduces sum|W| split DVE/ACT, ternary-quantizes
(clamp +-1.4999999 + round trick) and DMA-transposes into a resident bf16
[128, D/128, O] k-major rhs.

Alternate path (IMPL="v2", measured slower on HW despite better simulator
estimates; kept for reference):
- x sharded by tokens (16384/8 = 2048/core); W replicated (each core needs all
  of W quantized for its tokens' matmul).
- fp8 DoubleRow matmul: int8 activations a split exactly as a = 16h + l
  (h = round(a/16) in [-8,8], l in [-9,9]); 16h and l are exact in fp8e4m3,
  ternary weights exact in fp8. The PE DoubleRow perf mode contracts two
  adjacent 128-deep k-planes per instruction at 0.5 cycles/row, halving
  matmul time vs bf16; fp32 PSUM accumulation keeps everything bit-exact
  integer arithmetic, so the result equals the int8 reference matmul.
- All rounding via the +-1.5*2^23 magic-add trick (RNE). The h/l split is
  computed in five full-tile passes balanced across ACT (t2 = x*s/16+C,
  t1 = x*s+C), Pool/GPSIMD (h16 = 16*t2-16C, v = -16*t2+15C, l = t1+v), with
  the per-token absmax reduce on DVE (DVE passes are the HW bottleneck —
  measured ~2.5us/pass vs 1.2 ACT / 1.34 Pool for [128,2048]).
- Transposes on the PE (128x128 bf16 transpose via identity matmul into
  PSUM), with fp8-casting copybacks split ACT/DVE. No DMA transposes.
- W phase: single read (12 chunks resident + 4 streamed/re-read), sum|W|
  reduces split DVE/ACT, global partition reduce on GPSIMD, ternary
  quantize (clamp +-1.4999999 + round trick) on Pool/DVE, PE transposes
  into a resident fp8 [128, D/128, O] k-major rhs; DoubleRow pairs are
  adjacent k planes (no weight duplication). W loads ride the SWDGE queue,
  x loads the SP HWDGE queue, y stores alternate SP/ACT HWDGE queues.
- Main loop software-pipelined: quant stage A runs 2-3 token blocks ahead,
  transpose stage B one block ahead of its 16 DoubleRow matmuls (4 PSUM
  banks x 512 cols); scaled drains alternate ACT/DVE.
"""

import sys

sys.path.insert(0, "/opt/trn_rl_repo")

import numpy as np

import concourse.bacc as bacc
import concourse.bass as bass
import concourse.bass_isa as bass_isa
import concourse.mybir as mybir
import concourse.tile as tile

P = 128
MM_N = 512  # free-dim per matmul (one PSUM bank)
EXP23 = 12582912.0  # 1.5*2**23; v + C - C == RNE round for |v| < 2**22
EPS = 1e-8
CLIP_HI = 1.4999999  # largest f32 < 1.5; clamp-then-round == clip(round(.),-1,1)
N_CORES = 8
F32 = mybir.dt.float32
BF16 = mybir.dt.bfloat16
ALU = mybir.AluOpType
AFT = mybir.ActivationFunctionType
AX = mybir.AxisListType


def emit_bitlinear(
    tc: "tile.TileContext",
    y: "bass.AP",
    xs: "bass.AP",
    w: "bass.AP",
    repeat: int = 1,
    variant: str = "full",
):
    nc = tc.nc
    T, D = xs.shape
    O = w.shape[0]
    TB, NI, NR = T // P, D // P, O // P
    NOB = O // MM_N
    no_wphase = variant in ("no_wphase", "mm1")
    mm1 = variant == "mm1"

    from contextlib import ExitStack

    for _rep in range(repeat):
        _emit_bitlinear_once(tc, y, xs, w, variant, _rep)


def _emit_bitlinear_once(
    tc: "tile.TileContext",
    y: "bass.AP",
    xs: "bass.AP",
    w: "bass.AP",
    variant: str,
    rep: int = 0,
):
    nc = tc.nc
    T, D = xs.shape
    O = w.shape[0]
    TB, NI, NR = T // P, D // P, O // P
    NOB = O // MM_N
    no_wphase = variant in ("no_wphase", "mm1")
    mm1 = variant == "mm1"
    qsep = variant == "jouterq"  # W loads via SWDGE, y via HWDGE queues
    # jouter4: 4 early chains (needs one more streamed W chunk + aqte slots)
    NRES = NR - 4 if variant == "jouter4" else NR - 3

    from contextlib import ExitStack

    with ExitStack() as ctx:
        small = ctx.enter_context(tc.tile_pool(name=f"small{rep}", bufs=1))
        wqt_pool = ctx.enter_context(tc.tile_pool(name=f"wqt{rep}", bufs=1))
        ps_pool = ctx.enter_context(
            tc.tile_pool(name=f"psp{rep}", bufs=8, space="PSUM")
        )
        sc_pool = ctx.enter_context(tc.tile_pool(name=f"scp{rep}", bufs=4))
        # early-chain pools live across the whole kernel but are small; the
        # early aqT tiles must outlive the W phase (PE reads them later)
        aqte_pool = ctx.enter_context(tc.tile_pool(
            name=f"aqte{rep}", bufs=4 if variant == "jouter4" else 2))

        wqT = wqt_pool.tile([P, NI, O], BF16)
        w_scale = small.tile([P, 1], F32)

        def x_chain(tb, xpool_, aq_pool_, aqt_pool_):
            xt = xpool_.tile([P, D], F32, tag="x", name=f"x_{tb}")
            nc.sync.dma_start(xt[:], xs[tb * P : (tb + 1) * P, :])
            amax = sc_pool.tile([P, 1], F32, tag="amax", name=f"amax_{tb}")
            nc.vector.tensor_reduce(
                amax[:], xt[:], axis=AX.X, op=ALU.max, apply_absolute_value=True
            )
            a_eps = sc_pool.tile([P, 1], F32, tag="aeps", name=f"aeps_{tb}",
                                 bufs=TB)
            nc.vector.tensor_scalar_add(a_eps[:], amax[:], EPS)
            rcp = sc_pool.tile([P, 1], F32, tag="rcp", name=f"rcp_{tb}")
            nc.vector.reciprocal(rcp[:], a_eps[:])
            inv127 = sc_pool.tile([P, 1], F32, tag="i127", name=f"i127_{tb}")
            nc.vector.tensor_scalar_mul(inv127[:], rcp[:], 127.0)
            # round in place: xt <- xt*inv127 + C (ACT), then -C -> bf16 (DVE)
            nc.scalar.activation(
                xt[:], xt[:], AFT.Copy, bias=EXP23, scale=inv127[:]
            )
            aq = aq_pool_.tile([P, D], BF16, tag="aq", name=f"aq_{tb}")
            nc.vector.tensor_scalar_add(aq[:], xt[:], -EXP23)
            # aqT[p, j, t] = aq[t, j*128 + p]
            aqT = aqt_pool_.tile([P, NI, P], BF16, tag="aqT", name=f"aqT_{tb}")
            nc.scalar.dma_start(aqT[:], aq[:], transpose=True)
            return aqT, a_eps

        early = {}

        # ---- W phase: single read; NRES tiles resident, 3 streamed through
        # the xe pool, which also hosts two early token chains.
        if no_wphase:
            nc.gpsimd.memset(wqT[:], 1.0)
            nc.gpsimd.memset(w_scale[:], 0.01)
        else:
            with tc.tile_pool(name=f"wres{rep}", bufs=NRES) as wres, \
                 tc.tile_pool(name=f"wqrp{rep}", bufs=2) as wqrp, \
                 tc.tile_pool(name=f"xe{rep}", bufs=2) as xe_pool, \
                 tc.tile_pool(name=f"aqe{rep}", bufs=1) as aqe_pool:
                wsums = small.tile([P, NR], F32)
                wts = []

                def w_load(r, pool, tag, keep, name_pfx="wt"):
                    wt = pool.tile([P, D], F32, tag=tag,
                                   name=f"{name_pfx}_{r}")
                    weng = nc.gpsimd if qsep else nc.sync
                    weng.dma_start(wt[:], w[r * P : (r + 1) * P, :])
                    if keep:
                        wts.append(wt)
                        return wt
                    if r % 2 == 0:
                        nc.vector.tensor_reduce(
                            wsums[:, r : r + 1], wt[:], axis=AX.X, op=ALU.add,
                            apply_absolute_value=True,
                        )
                    else:
                        trash = wqrp.tile([P, D], BF16, tag="wqr",
                                          name=f"trash_{r}")
                        nc.scalar.activation(
                            trash[:], wt[:], AFT.Abs,
                            accum_out=wsums[:, r : r + 1],
                        )
                    wts.append(wt)
                    return wt

                for r in range(NRES):
                    w_load(r, wres, "wt", keep=False)
                # two early token chains through the xe/aqe pools; the last 3
                # W tiles are reduce-only here and re-read before quantize
                early[0] = x_chain(0, xe_pool, aqe_pool, aqte_pool)
                for r in range(NRES, NR):
                    wt = w_load(r, xe_pool, "x", keep=True, name_pfx="wta")
                    wts.pop()
                    if r % 2 == 0:
                        nc.vector.tensor_reduce(
                            wsums[:, r : r + 1], wt[:], axis=AX.X, op=ALU.add,
                            apply_absolute_value=True,
                        )
                    else:
                        trash = wqrp.tile([P, D], BF16, tag="wqr",
                                          name=f"trash_{r}")
                        nc.scalar.activation(
                            trash[:], wt[:], AFT.Abs,
                            accum_out=wsums[:, r : r + 1],
                        )
                    if r == NRES:
                        early[1] = x_chain(1, xe_pool, aqe_pool, aqte_pool)

                wsum1 = small.tile([P, 1], F32)
                nc.vector.tensor_reduce(
                    wsum1[:], wsums[:], axis=AX.X, op=ALU.add
                )
                wsum_all = small.tile([P, 1], F32)
                nc.gpsimd.partition_all_reduce(
                    wsum_all[:], wsum1[:], channels=P,
                    reduce_op=bass_isa.ReduceOp.add,
                )
                nc.vector.tensor_scalar(
                    w_scale[:], wsum_all[:], 1.0 / (O * D), EPS, op0=ALU.mult,
                    op1=ALU.add,
                )
                inv_w = small.tile([P, 1], F32)
                nc.vector.reciprocal(inv_w[:], w_scale[:])

                # re-read the streamed tiles now that w_scale is known
                for r in range(NRES, NR):
                    w_load(r, xe_pool, "x", keep=True, name_pfx="wtb")

                # in-place ternary quantize; streamed chunks first under
                # jouter4 so their xe slots free for the extra early chains
                if variant == "jouter4":
                    qorder = list(range(NRES, NR)) + list(range(NRES))
                else:
                    qorder = list(range(NR))
                for r in qorder:
                    u = wts[r]
                    eng = nc.gpsimd if r % 2 == 0 else nc.vector
                    eng.tensor_scalar(
                        u[:], u[:], inv_w[:], CLIP_HI, op0=ALU.mult,
                        op1=ALU.min,
                    )
                    eng.tensor_scalar(
                        u[:], u[:], -CLIP_HI, EXP23, op0=ALU.max, op1=ALU.add
                    )
                    wqr = wqrp.tile([P, D], BF16, tag="wqr", name=f"wqr_{r}")
                    if r % 2 == 0:
                        nc.scalar.activation(
                            wqr[:], u[:], AFT.Copy, bias=-EXP23
                        )
                    else:
                        nc.gpsimd.tensor_scalar_add(wqr[:], u[:], -EXP23)
                    nc.sync.dma_start(
                        wqT[:, :, r * P : (r + 1) * P], wqr[:], transpose=True
                    )
                    if variant == "jouter4" and r >= NR - 2:
                        k = 2 + (r - (NR - 2))
                        if k < TB and k not in early:
                            early[k] = x_chain(
                                k, xe_pool, aqe_pool, aqte_pool
                            )

        # ---- token pipeline pools (reuse the released W-phase SBUF).
        # "jouterslim" probes the pool-trim needed to free ~52KB/partition
        # for a future cross-rep W-prefetch ring (not shipped).
        slim = variant == "jouterslim"
        xpool = ctx.enter_context(
            tc.tile_pool(name=f"xp{rep}", bufs=3 if slim else 6))
        aq_pool = ctx.enter_context(
            tc.tile_pool(name=f"aqp{rep}", bufs=2 if slim else 4))
        aqt_pool = ctx.enter_context(
            tc.tile_pool(name=f"aqtp{rep}", bufs=4 if slim else 8))
        ypool = ctx.enter_context(
            tc.tile_pool(name=f"yp{rep}", bufs=4 if slim else 6))

        es_cache = {}

        def get_es(tb, a_eps):
            if tb not in es_cache:
                es = sc_pool.tile([P, 1], F32, tag="es", name=f"es_{tb}",
                                  bufs=8)
                nc.vector.tensor_scalar(
                    es[:], a_eps[:], w_scale[:], 1.0 / 127.0, op0=ALU.mult,
                    op1=ALU.mult,
                )
                es_cache[tb] = es
            return es_cache[tb]

        def drain(tb, ob, ps, es):
            ysb = ypool.tile([P, MM_N], F32, tag="y", name=f"y_{tb}_{ob}")
            nc.scalar.activation(ysb[:], ps[:], AFT.Copy, scale=es[:])
            if qsep:
                yeng = nc.sync if ob % 2 == 0 else nc.scalar
            else:
                yeng = nc.gpsimd
            yeng.dma_start(
                y[tb * P : (tb + 1) * P, ob * MM_N : (ob + 1) * MM_N], ysb[:]
            )

        def mm_quarter(tb, ob, aqT, a_eps):
            es = get_es(tb, a_eps)
            ps = ps_pool.tile([P, MM_N], F32, tag="ps", name=f"ps_{tb}_{ob}")
            NJ = 1 if mm1 else NI
            for j in range(NJ):
                nc.tensor.matmul(
                    ps[:],
                    lhsT=aqT[:, j, :],
                    rhs=wqT[:, j, ob * MM_N : (ob + 1) * MM_N],
                    start=(j == 0),
                    stop=(j == NJ - 1),
                )
            drain(tb, ob, ps, es)

        def mm_jouter(tb, aqT, a_eps):
            es = get_es(tb, a_eps)
            pss = [
                ps_pool.tile([P, MM_N], F32, tag="ps", name=f"ps_{tb}_{ob}")
                for ob in range(NOB)
            ]
            for j in range(NI):
                for ob in range(NOB):
                    nc.tensor.matmul(
                        pss[ob][:],
                        lhsT=aqT[:, j, :],
                        rhs=wqT[:, j, ob * MM_N : (ob + 1) * MM_N],
                        start=(j == 0),
                        stop=(j == NI - 1),
                    )
            for ob in range(NOB):
                drain(tb, ob, pss[ob], es)

        # ---- main loop. Hybrid ordering for j-outer variants: the first
        # two token blocks run ob-outer (their 16-matmul chains need only
        # wqT chunks 4ob..4ob+3, so the PE starts while later W chunks are
        # still quantizing/transposing — deps are range-tracked); steady
        # state runs j-outer for its accumulation-chain slack.
        jouter = variant in ("jouter", "jouterq", "jouter4", "jouterslim")
        for tb in range(TB):
            if tb in early:
                aqT, a_eps = early[tb]
            else:
                aqT, a_eps = x_chain(tb, xpool, aq_pool, aqt_pool)
            if jouter and tb >= 2:
                mm_jouter(tb, aqT, a_eps)
            else:
                for ob in range(NOB):
                    mm_quarter(tb, ob, aqT, a_eps)


def emit_bitlinear_v3(
    tc: "tile.TileContext",
    y: "bass.AP",
    xs: "bass.AP",
    w: "bass.AP",
    repeat: int = 1,
    variant: str = "v3",
):
    for rep in range(repeat):
        _emit_v3_once(tc, y, xs, w, variant, rep)


def _emit_v3_once(
    tc: "tile.TileContext",
    y: "bass.AP",
    xs: "bass.AP",
    w: "bass.AP",
    variant: str,
    rep: int = 0,
):
    """v1 restructured after HW calibration: the Pool engine's tensor_scalar
    is ~23us/pass (17x the cost model), so the W-phase quantize runs on
    ACT+DVE only: t = w*inv_w + C (ACT), clamp to [C-1, C+1] (DVE, exact:
    C +- k representable), cast-subtract C (ACT/DVE alternating). Queues:
    W loads -> SWDGE (gpsimd), W transposes -> sync, x loads -> sync,
    x transposes -> scalar, y stores -> gpsimd."""
    nc = tc.nc
    T, D = xs.shape
    O = w.shape[0]
    TB, NI, NR = T // P, D // P, O // P
    NOB = O // MM_N
    NRES = NR - 3
    no_wphase = variant in ("v3nw", "v3mm1")
    mm1 = variant == "v3mm1"
    wonly = variant.endswith("wonly")
    # W-load queue: gpsimd (SWDGE) default; ws = sync, wsc = scalar,
    # wh = alternate sync/scalar
    if variant.startswith("v3ws") and not variant.startswith("v3wsc"):
        wengs = ["sync"]
    elif variant.startswith("v3wsc"):
        wengs = ["scalar"]
    elif variant.startswith("v3wh"):
        wengs = ["sync", "scalar"]
    else:
        wengs = ["gpsimd"]

    from contextlib import ExitStack

    with ExitStack() as ctx:
        small = ctx.enter_context(tc.tile_pool(name=f"v3sm{rep}", bufs=1))
        wqt_pool = ctx.enter_context(tc.tile_pool(name=f"v3wq{rep}", bufs=1))
        ps_pool = ctx.enter_context(
            tc.tile_pool(name=f"v3ps{rep}", bufs=8, space="PSUM")
        )
        sc_pool = ctx.enter_context(tc.tile_pool(name=f"v3sc{rep}", bufs=4))
        aqte_pool = ctx.enter_context(tc.tile_pool(name=f"v3ae{rep}", bufs=2))

        wqT = wqt_pool.tile([P, NI, O], BF16)
        w_scale = small.tile([P, 1], F32)

        def x_chain(tb, xpool_, aq_pool_, aqt_pool_):
            xt = xpool_.tile([P, D], F32, tag="x", name=f"x_{tb}")
            nc.sync.dma_start(xt[:], xs[tb * P : (tb + 1) * P, :])
            amax = sc_pool.tile([P, 1], F32, tag="amax", name=f"amax_{tb}")
            nc.vector.tensor_reduce(
                amax[:], xt[:], axis=AX.X, op=ALU.max, apply_absolute_value=True
            )
            a_eps = sc_pool.tile([P, 1], F32, tag="aeps", name=f"aeps_{tb}",
                                 bufs=TB)
            nc.vector.tensor_scalar_add(a_eps[:], amax[:], EPS)
            rcp = sc_pool.tile([P, 1], F32, tag="rcp", name=f"rcp_{tb}")
            nc.vector.reciprocal(rcp[:], a_eps[:])
            inv127 = sc_pool.tile([P, 1], F32, tag="i127", name=f"i127_{tb}")
            nc.vector.tensor_scalar_mul(inv127[:], rcp[:], 127.0)
            # round in place: xt <- xt*inv127 + C (ACT), then -C -> bf16
            # (alternate ACT/DVE per block to balance engine load)
            nc.scalar.activation(
                xt[:], xt[:], AFT.Copy, bias=EXP23, scale=inv127[:]
            )
            aq = aq_pool_.tile([P, D], BF16, tag="aq", name=f"aq_{tb}")
            if tb % 2 == 0:
                nc.scalar.activation(aq[:], xt[:], AFT.Copy, bias=-EXP23)
            else:
                nc.vector.tensor_scalar_add(aq[:], xt[:], -EXP23)
            # aqT[p, j, t] = aq[t, j*128 + p]
            aqT = aqt_pool_.tile([P, NI, P], BF16, tag="aqT", name=f"aqT_{tb}")
            nc.scalar.dma_start(aqT[:], aq[:], transpose=True)
            return aqT, a_eps

        early = {}

        # ---- W phase: loads on SWDGE; reduces DVE/ACT; quantize ACT+DVE.
        if no_wphase:
            for j in range(NI):
                eng = nc.vector if j % 2 == 0 else nc.scalar
                if j % 2 == 0:
                    eng.memset(wqT[:, j, :], 1.0)
                else:
                    eng.memset(wqT[:, j, :], 1.0)
            nc.vector.memset(w_scale[:], 0.01)
        else:
            with tc.tile_pool(name=f"v3wr{rep}", bufs=NRES) as wres, \
                 tc.tile_pool(name=f"v3qr{rep}", bufs=2) as wqrp, \
                 tc.tile_pool(name=f"v3xe{rep}", bufs=2) as xe_pool, \
                 tc.tile_pool(name=f"v3qe{rep}", bufs=1) as aqe_pool:
                wsums = small.tile([P, NR], F32)
                wts = []

                def w_reduce(r, wt):
                    if r % 2 == 0:
                        nc.vector.tensor_reduce(
                            wsums[:, r : r + 1], wt[:], axis=AX.X, op=ALU.add,
                            apply_absolute_value=True,
                        )
                    else:
                        trash = wqrp.tile([P, D], BF16, tag="wqr",
                                          name=f"trash_{r}")
                        nc.scalar.activation(
                            trash[:], wt[:], AFT.Abs,
                            accum_out=wsums[:, r : r + 1],
                        )

                def w_eng(r):
                    return getattr(nc, wengs[r % len(wengs)])

                for r in range(NRES):
                    wt = wres.tile([P, D], F32, tag="wt", name=f"wt_{r}")
                    w_eng(r).dma_start(wt[:], w[r * P : (r + 1) * P, :])
                    w_reduce(r, wt)
                    wts.append(wt)
                early[0] = x_chain(0, xe_pool, aqe_pool, aqte_pool)
                for r in range(NRES, NR):
                    wt = xe_pool.tile([P, D], F32, tag="x", name=f"wta_{r}")
                    w_eng(r).dma_start(wt[:], w[r * P : (r + 1) * P, :])
                    w_reduce(r, wt)
                    if r == NRES:
                        early[1] = x_chain(1, xe_pool, aqe_pool, aqte_pool)

                wsum1 = small.tile([P, 1], F32)
                nc.vector.tensor_reduce(
                    wsum1[:], wsums[:], axis=AX.X, op=ALU.add
                )
                wsum_all = small.tile([P, 1], F32)
                nc.gpsimd.partition_all_reduce(
                    wsum_all[:], wsum1[:], channels=P,
                    reduce_op=bass_isa.ReduceOp.add,
                )
                nc.vector.tensor_scalar(
                    w_scale[:], wsum_all[:], 1.0 / (O * D), EPS, op0=ALU.mult,
                    op1=ALU.add,
                )
                inv_w = small.tile([P, 1], F32)
                nc.vector.reciprocal(inv_w[:], w_scale[:])

                # re-read the streamed tiles now that w_scale is known
                for r in range(NRES, NR):
                    wt = xe_pool.tile([P, D], F32, tag="x", name=f"wtb_{r}")
                    w_eng(r).dma_start(wt[:], w[r * P : (r + 1) * P, :])
                    wts.append(wt)

                # quantize: ACT magic-add round, DVE clamp, ACT/DVE cast-sub
                for r in range(NR):
                    u = wts[r]
                    nc.scalar.activation(
                        u[:], u[:], AFT.Copy, bias=EXP23, scale=inv_w[:]
                    )
                    nc.vector.tensor_scalar(
                        u[:], u[:], EXP23 - 1.0, EXP23 + 1.0, op0=ALU.max,
                        op1=ALU.min,
                    )
                    wqr = wqrp.tile([P, D], BF16, tag="wqr", name=f"wqr_{r}")
                    if r % 2 == 0:
                        nc.scalar.activation(
                            wqr[:], u[:], AFT.Copy, bias=-EXP23
                        )
                    else:
                        nc.vector.tensor_scalar_add(wqr[:], u[:], -EXP23)
                    nc.sync.dma_start(
                        wqT[:, :, r * P : (r + 1) * P], wqr[:], transpose=True
                    )

        if wonly:
            return

        # ---- token pipeline pools (reuse the released W-phase SBUF)
        xpool = ctx.enter_context(tc.tile_pool(name=f"v3xp{rep}", bufs=6))
        aq_pool = ctx.enter_context(tc.tile_pool(name=f"v3ap{rep}", bufs=4))
        aqt_pool = ctx.enter_context(tc.tile_pool(name=f"v3tp{rep}", bufs=8))
        ypool = ctx.enter_context(tc.tile_pool(name=f"v3yp{rep}", bufs=6))

        es_cache = {}

        def get_es(tb, a_eps):
            if tb not in es_cache:
                es = sc_pool.tile([P, 1], F32, tag="es", name=f"es_{tb}",
                                  bufs=8)
                nc.vector.tensor_scalar(
                    es[:], a_eps[:], w_scale[:], 1.0 / 127.0, op0=ALU.mult,
                    op1=ALU.mult,
                )
                es_cache[tb] = es
            return es_cache[tb]

        def drain(tb, ob, ps, es):
            ysb = ypool.tile([P, MM_N], F32, tag="y", name=f"y_{tb}_{ob}")
            nc.scalar.activation(ysb[:], ps[:], AFT.Copy, scale=es[:])
            nc.gpsimd.dma_start(
                y[tb * P : (tb + 1) * P, ob * MM_N : (ob + 1) * MM_N], ysb[:]
            )

        def mm_quarter(tb, ob, aqT, a_eps):
            es = get_es(tb, a_eps)
            ps = ps_pool.tile([P, MM_N], F32, tag="ps", name=f"ps_{tb}_{ob}")
            NJ = 1 if mm1 else NI
            for j in range(NJ):
                nc.tensor.matmul(
                    ps[:],
                    lhsT=aqT[:, j, :],
                    rhs=wqT[:, j, ob * MM_N : (ob + 1) * MM_N],
                    start=(j == 0),
                    stop=(j == NJ - 1),
                )
            drain(tb, ob, ps, es)

        def mm_jouter(tb, aqT, a_eps):
            es = get_es(tb, a_eps)
            pss = [
                ps_pool.tile([P, MM_N], F32, tag="ps", name=f"ps_{tb}_{ob}")
                for ob in range(NOB)
            ]
            for j in range(NI):
                for ob in range(NOB):
                    nc.tensor.matmul(
                        pss[ob][:],
                        lhsT=aqT[:, j, :],
                        rhs=wqT[:, j, ob * MM_N : (ob + 1) * MM_N],
                        start=(j == 0),
                        stop=(j == NI - 1),
                    )
            for ob in range(NOB):
                drain(tb, ob, pss[ob], es)

        for tb in range(TB):
            if tb in early:
                aqT, a_eps = early[tb]
            else:
                aqT, a_eps = x_chain(tb, xpool, aq_pool, aqt_pool)
            if tb >= 2 and not mm1:
                mm_jouter(tb, aqT, a_eps)
            else:
                for ob in range(NOB):
                    mm_quarter(tb, ob, aqT, a_eps)


FP8 = mybir.dt.float8e4
EXP23_16 = 16.0 * EXP23
EXP23_15 = 15.0 * EXP23


def emit_bitlinear_v2(
    tc: "tile.TileContext",
    y: "bass.AP",
    xs: "bass.AP",
    w: "bass.AP",
    repeat: int = 1,
    variant: str = "full",
    mm: str = "dr",
):
    for rep in range(repeat):
        _emit_v2_once(tc, y, xs, w, variant, mm, rep)


def _emit_v2_once(
    tc: "tile.TileContext",
    y: "bass.AP",
    xs: "bass.AP",
    w: "bass.AP",
    variant: str,
    mm: str,
    rep: int,
):
    from contextlib import ExitStack

    from concourse import masks

    nc = tc.nc
    T, D = xs.shape
    O = w.shape[0]
    TB, NI, NR = T // P, D // P, O // P
    NOB = O // MM_N
    dr = mm == "dr"
    mm1 = variant == "mm1"
    NRES = NR - 4  # W tiles kept resident; rest streamed + re-read
    N_EARLY = min(4, TB)

    with ExitStack() as ctx:
        small = ctx.enter_context(tc.tile_pool(name=f"v2small{rep}", bufs=1))
        wq_pool = ctx.enter_context(tc.tile_pool(name=f"v2wq{rep}", bufs=1))
        ps_mm = ctx.enter_context(
            tc.tile_pool(name=f"v2psmm{rep}", bufs=4, space="PSUM")
        )
        ps_tr = ctx.enter_context(
            tc.tile_pool(name=f"v2pstr{rep}", bufs=3, space="PSUM")
        )
        sc_pool = ctx.enter_context(tc.tile_pool(name=f"v2scp{rep}", bufs=4))
        aqte_pool = ctx.enter_context(
            tc.tile_pool(name=f"v2aqte{rep}", bufs=N_EARLY)
        )

        ident = small.tile([P, P], BF16)
        masks.make_identity(nc, ident[:])
        # wq8: ternary weights, k-major [P(i), NI(j), O]; DR pairs are
        # adjacent j planes. bf16 path stores bf16 instead.
        wq8 = wq_pool.tile([P, NI, O], FP8 if dr else BF16)
        w_scale = small.tile([P, 1], F32)

        def x_stage_a(tb, xpool_, t2pool_, hlpool_):
            """DMA + absmax + quant passes; returns tiles for stage b."""
            xt = xpool_.tile([P, D], F32, tag="x", name=f"x_{tb}")
            nc.sync.dma_start(xt[:], xs[tb * P : (tb + 1) * P, :])
            amax = sc_pool.tile([P, 1], F32, tag="amax", name=f"amax_{tb}")
            nc.vector.tensor_reduce(
                amax[:], xt[:], axis=AX.X, op=ALU.max, apply_absolute_value=True
            )
            a_eps = sc_pool.tile(
                [P, 1], F32, tag="aeps", name=f"aeps_{tb}", bufs=TB
            )
            nc.vector.tensor_scalar_add(a_eps[:], amax[:], EPS)
            rcp = sc_pool.tile([P, 1], F32, tag="rcp", name=f"rcp_{tb}")
            nc.vector.reciprocal(rcp[:], a_eps[:])
            s = sc_pool.tile([P, 1], F32, tag="s", name=f"s_{tb}")
            nc.vector.tensor_scalar_mul(s[:], rcp[:], 127.0)
            if dr:
                s16 = sc_pool.tile([P, 1], F32, tag="s16", name=f"s16_{tb}")
                nc.vector.tensor_scalar_mul(s16[:], rcp[:], 127.0 / 16.0)
                # t2 = x*s/16 + C  (== C + h exactly, h = round(a/16))
                t2 = t2pool_.tile([P, D], F32, tag="t2", name=f"t2_{tb}")
                nc.scalar.activation(
                    t2[:], xt[:], AFT.Copy, bias=EXP23, scale=s16[:]
                )
                # t1 = x*s + C in place (== C + a exactly)
                nc.scalar.activation(
                    xt[:], xt[:], AFT.Copy, bias=EXP23, scale=s[:]
                )
                # h16 = 16h = 16*t2 - 16C (bf16); on Pool to offload ACT/DVE
                h16 = hlpool_.tile([P, D], BF16, tag="h", name=f"h_{tb}")
                nc.gpsimd.tensor_scalar(
                    h16[:], t2[:], 16.0, -EXP23_16, op0=ALU.mult, op1=ALU.add
                )
                # v = -16*t2 + 15C = -C - 16h, in place over t2 (Pool)
                nc.gpsimd.tensor_scalar(
                    t2[:], t2[:], -16.0, EXP23_15, op0=ALU.mult, op1=ALU.add
                )
                # l = t1 + v = a - 16h (bf16) (Pool)
                l16 = hlpool_.tile([P, D], BF16, tag="l", name=f"l_{tb}")
                nc.gpsimd.tensor_tensor(l16[:], xt[:], t2[:], op=ALU.add)
                return (tb, ((0, h16), (1, l16)), a_eps, 2)
            else:
                nc.scalar.activation(
                    xt[:], xt[:], AFT.Copy, bias=EXP23, scale=s[:]
                )
                aq = hlpool_.tile([P, D], BF16, tag="h", name=f"aq_{tb}")
                nc.gpsimd.tensor_scalar_add(aq[:], xt[:], -EXP23)
                return (tb, ((0, aq),), a_eps, 1)

        def x_stage_b(st, aqt_pool_):
            """PE transposes (bf16) -> psum -> cast copyback (rotated)."""
            tb, srcs, a_eps, npair = st
            aqT = aqt_pool_.tile(
                [P, npair, NI, P], FP8 if dr else BF16, tag="aqT",
                name=f"aqT_{tb}",
            )
            cb_engs = (nc.scalar, nc.vector, nc.scalar, nc.vector)
            for half, src in srcs:
                for g in range(2):
                    pst = ps_tr.tile(
                        [P, 8 * P], BF16, tag="pst",
                        name=f"pst_{tb}_{half}_{g}",
                    )
                    for k in range(8):
                        j = g * 8 + k
                        nc.tensor.transpose(
                            pst[:, k * P : (k + 1) * P],
                            src[:, j * P : (j + 1) * P],
                            ident[:],
                        )
                    eng = cb_engs[(half * 2 + g) % len(cb_engs)]
                    dst = aqT[:, half, g * 8 : (g + 1) * 8, :]
                    if eng is nc.scalar:
                        nc.scalar.activation(dst, pst[:], AFT.Copy)
                    else:
                        eng.tensor_copy(dst, pst[:])
            return aqT, a_eps

        stage_a = {}
        done_b = {}

        # ---- W phase (W loads on the scalar DMA queue; x on sync queue)
        if variant == "no_wphase":
            nc.vector.memset(wq8[:], 1.0)
            nc.vector.memset(w_scale[:], 0.01)
        else:
            with tc.tile_pool(name=f"v2wres{rep}", bufs=NRES) as wres, \
                 tc.tile_pool(name=f"v2wqr{rep}", bufs=2) as wqrp, \
                 tc.tile_pool(name=f"v2xe{rep}", bufs=3) as xe_pool, \
                 tc.tile_pool(name=f"v2ep{rep}", bufs=1) as early_pool:
                wsums = small.tile([P, NR], F32)
                wts = []

                def w_reduce(r, wt):
                    if r % 2 == 0:
                        nc.vector.tensor_reduce(
                            wsums[:, r : r + 1], wt[:], axis=AX.X, op=ALU.add,
                            apply_absolute_value=True,
                        )
                    else:
                        trash = wqrp.tile([P, D], BF16, tag="wqr",
                                          name=f"trash_{r}")
                        nc.scalar.activation(
                            trash[:], wt[:], AFT.Abs,
                            accum_out=wsums[:, r : r + 1],
                        )

                n_a = 0
                for r in range(NRES):
                    wt = wres.tile([P, D], F32, tag="wt", name=f"wt_{r}")
                    nc.gpsimd.dma_start(wt[:], w[r * P : (r + 1) * P, :])
                    w_reduce(r, wt)
                    wts.append(wt)
                    if r % 3 == 2 and n_a < N_EARLY:
                        stage_a[n_a] = x_stage_a(
                            n_a, early_pool, early_pool, early_pool
                        )
                        n_a += 1
                for r in range(NRES, NR):
                    wt = xe_pool.tile([P, D], F32, tag="x", name=f"wta_{r}")
                    nc.gpsimd.dma_start(wt[:], w[r * P : (r + 1) * P, :])
                    w_reduce(r, wt)
                while n_a < N_EARLY:
                    stage_a[n_a] = x_stage_a(
                        n_a, early_pool, early_pool, early_pool
                    )
                    n_a += 1

                wsum1 = small.tile([P, 1], F32)
                nc.vector.tensor_reduce(
                    wsum1[:], wsums[:], axis=AX.X, op=ALU.add
                )
                wsum_all = small.tile([P, 1], F32)
                nc.gpsimd.partition_all_reduce(
                    wsum_all[:], wsum1[:], channels=P,
                    reduce_op=bass_isa.ReduceOp.add,
                )
                nc.vector.tensor_scalar(
                    w_scale[:], wsum_all[:], 1.0 / (O * D), EPS, op0=ALU.mult,
                    op1=ALU.add,
                )
                inv_w = small.tile([P, 1], F32)
                nc.vector.reciprocal(inv_w[:], w_scale[:])

                done_b[0] = x_stage_b(stage_a.pop(0), aqte_pool)

                for r in range(NRES, NR):
                    wt = xe_pool.tile([P, D], F32, tag="x", name=f"wtb_{r}")
                    nc.gpsimd.dma_start(wt[:], w[r * P : (r + 1) * P, :])
                    wts.append(wt)

                for r in range(NR):
                    u = wts[r]
                    eng = nc.gpsimd if r % 2 == 0 else nc.vector
                    eng.tensor_scalar(
                        u[:], u[:], inv_w[:], CLIP_HI, op0=ALU.mult,
                        op1=ALU.min,
                    )
                    eng.tensor_scalar(
                        u[:], u[:], -CLIP_HI, EXP23, op0=ALU.max, op1=ALU.add
                    )
                    wqr = wqrp.tile([P, D], BF16, tag="wqr", name=f"wqr_{r}")
                    if r % 2 == 0:
                        nc.scalar.activation(
                            wqr[:], u[:], AFT.Copy, bias=-EXP23
                        )
                    else:
                        nc.gpsimd.tensor_scalar_add(wqr[:], u[:], -EXP23)
                    # PE transpose wqr [o,i] -> wq8[:, :, r*128...] (i major)
                    for g in range(2):
                        pst = ps_tr.tile(
                            [P, 8 * P], BF16, tag="pst",
                            name=f"wpst_{r}_{g}",
                        )
                        for k in range(8):
                            j = g * 8 + k
                            nc.tensor.transpose(
                                pst[:, k * P : (k + 1) * P],
                                wqr[:, j * P : (j + 1) * P],
                                ident[:],
                            )
                        dst = wq8[:, g * 8 : (g + 1) * 8, r * P : (r + 1) * P]
                        src8 = pst[:].rearrange("p (j o) -> p j o", j=8)
                        if r % 2 == 0:
                            nc.vector.tensor_copy(dst, src8)
                        else:
                            nc.scalar.activation(dst, src8, AFT.Copy)
                    if r % 4 == 1 and (r // 4 + 1) < N_EARLY:
                        k = r // 4 + 1
                        done_b[k] = x_stage_b(stage_a.pop(k), aqte_pool)
                while stage_a:
                    k = min(stage_a)
                    done_b[k] = x_stage_b(stage_a.pop(k), aqte_pool)

        # ---- main token pipeline (software pipelined: A two ahead,
        # B one ahead of the matmul consumer)
        xpool = ctx.enter_context(tc.tile_pool(name=f"v2xp{rep}", bufs=4))
        t2pool = ctx.enter_context(tc.tile_pool(name=f"v2t2{rep}", bufs=4))
        hlpool = ctx.enter_context(tc.tile_pool(name=f"v2hl{rep}", bufs=4))
        aqt_pool = ctx.enter_context(tc.tile_pool(name=f"v2aqt{rep}", bufs=4))
        ypool = ctx.enter_context(tc.tile_pool(name=f"v2yp{rep}", bufs=6))

        es_cache = {}

        def mm_quarter(tb, ob, aqT, a_eps):
            if tb not in es_cache:
                es = sc_pool.tile([P, 1], F32, tag="es", name=f"es_{tb}",
                                  bufs=8)
                nc.vector.tensor_scalar(
                    es[:], a_eps[:], w_scale[:], 1.0 / 127.0, op0=ALU.mult,
                    op1=ALU.mult,
                )
                es_cache[tb] = es
            es = es_cache[tb]
            ps = ps_mm.tile([P, MM_N], F32, tag="ps", name=f"ps_{tb}_{ob}")
            sl = slice(ob * MM_N, (ob + 1) * MM_N)
            if dr:
                NJP = 1 if mm1 else NI // 2
                for half in range(2):
                    for jp in range(NJP):
                        nc.tensor.matmul(
                            ps[:],
                            lhsT=aqT[:, half, 2 * jp : 2 * jp + 2, :],
                            rhs=wq8[:, 2 * jp : 2 * jp + 2, sl],
                            start=(half == 0 and jp == 0),
                            stop=(half == 1 and jp == NJP - 1),
                            perf_mode=mybir.MatmulPerfMode.DoubleRow,
                        )
            else:
                NJ = 1 if mm1 else NI
                for j in range(NJ):
                    nc.tensor.matmul(
                        ps[:],
                        lhsT=aqT[:, 0, j, :],
                        rhs=wq8[:, j, sl],
                        start=(j == 0),
                        stop=(j == NJ - 1),
                    )
            ysb = ypool.tile([P, MM_N], F32, tag="y", name=f"y_{tb}_{ob}")
            if ob % 2 == 0:
                nc.scalar.activation(ysb[:], ps[:], AFT.Copy, scale=es[:])
            else:
                nc.vector.tensor_scalar_mul(ysb[:], ps[:], es[:])
            yeng = nc.sync if ob % 2 == 0 else nc.scalar
            yeng.dma_start(
                y[tb * P : (tb + 1) * P, ob * MM_N : (ob + 1) * MM_N], ysb[:]
            )

        for tb in range(TB):
            for ta in (tb + 2, tb + 3):
                if ta < TB and ta not in done_b and ta not in stage_a:
                    stage_a[ta] = x_stage_a(ta, xpool, t2pool, hlpool)
            tbb = tb + 1
            if tbb < TB and tbb not in done_b:
                done_b[tbb] = x_stage_b(stage_a.pop(tbb), aqt_pool)
            if tb not in done_b:
                done_b[tb] = x_stage_b(stage_a.pop(tb), aqt_pool)
            aqT, a_eps = done_b[tb]
            for ob in range(NOB):
                mm_quarter(tb, ob, aqT, a_eps)
            del done_b[tb]
_NC_CACHE: dict = {}


def _get_nc(
    T: int,
    D: int,
    O: int,
    repeat: int = 1,
    variant: str | None = None,
    impl: str | None = None,
    mm: str = "dr",
) -> "bass.Bass":
    if impl is None:
        impl = IMPL
    if variant is None:
        variant = {"v1": V1_VARIANT, "v3": V3_VARIANT}.get(impl, "full")
    key = (T, D, O, repeat, variant, impl, mm)
    if key not in _NC_CACHE:
        nc = bacc.Bacc("TRN2", target_bir_lowering=False, debug=False)
        xs = nc.dram_tensor("xs", [T, D], F32, kind="ExternalInput").ap()
        w = nc.dram_tensor("w", [O, D], F32, kind="ExternalInput").ap()
        y = nc.dram_tensor("y", [T, O], F32, kind="ExternalOutput").ap()
        with tile.TileContext(nc) as tc:
            if impl == "v1":
                emit_bitlinear(tc, y, xs, w, repeat=repeat, variant=variant)
            elif impl == "v3":
                emit_bitlinear_v3(
                    tc, y, xs, w, repeat=repeat, variant=variant
                )
            else:
                emit_bitlinear_v2(
                    tc, y, xs, w, repeat=repeat, variant=variant, mm=mm
                )
        nc.compile()
        _NC_CACHE[key] = nc
    return _NC_CACHE[key]


IMPL = "v3"
MM = "dr"
V1_VARIANT = "jouter"
V3_VARIANT = "v3ws"


def kernel(
    x: np.ndarray, weight: np.ndarray, _trace: bool = False, _repeat: int = 1
):
    from concourse.bass_utils import run_bass_kernel_spmd

    x = np.asarray(x, dtype=np.float32)
    weight = np.ascontiguousarray(np.asarray(weight, dtype=np.float32))
    B, S, D = x.shape
    O = weight.shape[0]
    tokens = B * S
    Tc = tokens // N_CORES
    xf = np.ascontiguousarray(x.reshape(tokens, D))

    nc = _get_nc(Tc, D, O, repeat=_repeat, impl=IMPL, mm=MM)
    in_maps = [
        {"xs": np.ascontiguousarray(xf[c * Tc : (c + 1) * Tc]), "w": weight}
        for c in range(N_CORES)
    ]
    res = run_bass_kernel_spmd(
        nc, in_maps, list(range(N_CORES)), trace=_trace
    )
    out = np.concatenate([res.results[c]["y"] for c in range(N_CORES)], axis=0)
    out = out.reshape(B, S, O)
    if _trace:
        return out, res
    return out

